# revision 1
# baseline (speedup 1.0000x reference)
"""Trainium2 Bass/Tile kernel: 3x3 conv (zero pad) + bias - theta * cross-stencil
(replicate pad) over NCHW f32, B=32, Cin=Cout=128, H=W=128, theta=0.7.

Math: the stencil term is a 3x3 conv with kernel [[0,1,0],[1,-4,1],[0,1,0]]
applied to sum_ci(x), identical for all (out,in) channel pairs.  For interior
pixels it folds into the conv weights:  W' = W - theta*cross.  The only
difference is at the 1-pixel image border where the stencil uses replicate
padding (out-of-bounds neighbor == edge value) while the conv uses zero
padding.  So:  out = conv_zp(x, W') + b - theta*corr, where corr adds
s=sum_ci(x) at each border pixel once per out-of-bounds neighbor (corners
twice).  corr is computed on-device from four border-strip channel sums
(ones-vector matmuls) broadcast across the 128 output channels.

Sharding: data-parallel over batch, 4 images per core, 8 cores, SPMD.
"""

import os
import numpy as np

THETA = 0.7
N_CORES = 8
B, CIN, COUT, H, W = 32, 128, 128, 128, 128
BL = B // N_CORES          # images per core
PW = W + 2                 # horizontally padded row width in SBUF
ROWS_PER_BLK = 16          # output rows per SBUF staging tile / output DMA
CHUNK = 4                  # output rows per PSUM accumulation group (N=512)

COMPUTE = os.environ.get("BASS_CONV_DTYPE", "f32r")  # "f32r" | "bf16"

_built = {}
_runner = {}


def _patch_tile_drain():
    """This toolchain's walrus rejects instructions carrying more than one
    semaphore wait ('Too many sync wait commands' in setupSyncWait).  Tile's
    exit drain accumulates one wait per live semaphore on a single Drain, so
    re-emit those waits as a chain of single-wait NOPs in front of it."""
    import concourse.tile as tile
    import concourse.mybir as mybir
    from concourse.vector_clock import ScopedClock

    if getattr(tile.TileContext, "_drain_patched", False):
        return

    def _drain_and_barrier(self, tick_clock, wait_clock):
        nc = self.nc
        probe = nc.sync.nop(nofuse=True)
        wait_clock.add_sem_waits(
            probe.ins, ScopedClock({None: tick_clock.global_clock})
        )
        si = probe.ins.sync_info
        waits = list(si.on_wait) if si is not None and si.on_wait else []
        if len(waits) > 1:
            si.on_wait = waits[:1]
            for w in waits[1:]:
                nop = nc.sync.nop(nofuse=True)
                if nop.ins.sync_info is None:
                    nop.ins.sync_info = mybir.SyncInfo(on_wait=[w], on_update=[])
                else:
                    nop.ins.sync_info.on_wait = [w]
        nc.sync.drain()

        nc.all_engine_barrier()
        assert self.sems is not None
        popped = nc._tile_sem_poison_stack.pop()
        assert popped is self._sem_poison
        nc.clear_and_free_semaphores(list(self.sems.allocated().values()))
        nc.all_engine_barrier()

    tile.TileContext._drain_and_barrier = _drain_and_barrier
    tile.TileContext._drain_patched = True


def _split_excess_waits(nc, cap=1):
    """Hoist extra semaphore waits (walrus allows only `cap` per instruction
    on this toolchain) onto same-engine NOPs inserted just before the
    offending instruction."""
    import concourse.mybir as mybir

    n = 0
    for bb in nc.main_func.blocks:
        insts = bb.instructions
        out = []
        for inst in insts:
            si = inst.sync_info
            waits = list(si.on_wait) if si is not None and si.on_wait else []
            if len(waits) > cap:
                n += 1
                for i in range(0, len(waits) - cap, cap):
                    chunk = waits[i : i + cap]
                    nop = mybir.InstNoOp(
                        name=nc.get_next_instruction_name(),
                        sync_info=mybir.SyncInfo(on_wait=list(chunk), on_update=[]),
                        engine=inst.engine,
                        bass_nofuse=True,
                    )
                    nc.register_instruction(nop)
                    out.append(nop)
                si.on_wait = waits[len(waits) - cap :]
            out.append(inst)
        insts[:] = out
    return n


def _build(compute, reps=1):
    import concourse.bass as bass
    import concourse.mybir as mybir
    import concourse.tile as tile

    _patch_tile_drain()
    cdt = {"f32r": mybir.dt.float32r, "bf16": mybir.dt.bfloat16}[compute]
    f32 = mybir.dt.float32
    AF = mybir.ActivationFunctionType

    nc = bass.Bass()
    # x arrives pre-padded on the host: two zero columns per row (PW=W+2),
    # so the image DMA is one contiguous copy and zero-padding needs no
    # on-device memsets (f32r memset fails the walrus ISA check).
    x_ext = nc.declare_dram_parameter("x", [BL, CIN, H, PW], cdt, isOutput=False)
    wt_ext = nc.declare_dram_parameter("Wt", [CIN, 9 * COUT], cdt, isOutput=False)
    b_ext = nc.declare_dram_parameter("bias", [COUT, 1], f32, isOutput=False)
    ones_ext = nc.declare_dram_parameter("ones", [CIN, 1], cdt, isOutput=False)
    y_ext = nc.declare_dram_parameter("y", [BL, COUT, H, W], f32, isOutput=True)

    NBLK = H // ROWS_PER_BLK
    NCH = ROWS_PER_BLK // CHUNK

    with tile.TileContext(nc) as tc:
        with (
            tc.tile_pool(name="singles", bufs=1) as singles,
            tc.tile_pool(name="xin", bufs=2) as xpool,
            tc.tile_pool(name="outs", bufs=3) as outpool,
            tc.tile_pool(name="corr", bufs=2) as corrpool,
            tc.tile_pool(name="pmm", bufs=4, space="PSUM") as pmm,
            tc.tile_pool(name="psmall", bufs=2, space="PSUM") as psmall,
            tc.tile_pool(name="pbcast", bufs=2, space="PSUM") as pbcast,
        ):
            w_sb = singles.tile([CIN, 9 * COUT], cdt)
            nc.sync.dma_start(out=w_sb, in_=wt_ext[:])
            bias_sb = singles.tile([COUT, 1], f32)
            nc.sync.dma_start(out=bias_sb, in_=b_ext[:])
            ones_col = singles.tile([CIN, 1], cdt)
            nc.sync.dma_start(out=ones_col, in_=ones_ext[:])
            ones_row = singles.tile([1, COUT], f32)
            nc.vector.memset(ones_row, 1.0)

            for img_rep in range(BL * reps):
                img = img_rep % BL
                x_t = xpool.tile([CIN, H, PW], cdt)
                nc.sync.dma_start(out=x_t, in_=x_ext[img])

                # Channel sums of the four border strips -> one PSUM bank:
                # [0:H) left col, [H:2H) right col, [2H:2H+W) top, [2H+W:) bottom
                ps_s = psmall.tile([1, 2 * H + 2 * W], f32)
                nc.tensor.matmul(
                    ps_s[:, 0:H], ones_col, x_t[:, :, 1:2], start=True, stop=False
                )
                nc.tensor.matmul(
                    ps_s[:, H : 2 * H], ones_col, x_t[:, :, W : W + 1],
                    start=False, stop=False,
                )
                nc.tensor.matmul(
                    ps_s[:, 2 * H : 2 * H + W], ones_col, x_t[:, 0:1, 1 : W + 1],
                    start=False, stop=False,
                )
                nc.tensor.matmul(
                    ps_s[:, 2 * H + W :], ones_col, x_t[:, H - 1 : H, 1 : W + 1],
                    start=False, stop=True,
                )
                s_sb = corrpool.tile([1, 2 * H + 2 * W], f32)
                nc.scalar.activation(out=s_sb, in_=ps_s, func=AF.Copy)
                # broadcast across the 128 out-channel partitions, scaled by theta
                ps_c = pbcast.tile([COUT, 2 * H + 2 * W], f32)
                nc.tensor.matmul(ps_c, ones_row, s_sb, start=True, stop=True)
                corr_sb = corrpool.tile([COUT, 2 * H + 2 * W], f32)
                nc.scalar.activation(out=corr_sb, in_=ps_c, func=AF.Copy, scale=THETA)

                for blk in range(NBLK):
                    out_sb = outpool.tile([COUT, ROWS_PER_BLK * W], f32)
                    for q in range(NCH):
                        y0 = blk * ROWS_PER_BLK + q * CHUNK
                        ps = pmm.tile([COUT, CHUNK * W], f32)
                        first = True
                        for ky in range(3):
                            r0 = y0 + ky - 1
                            rows, out_off = CHUNK, 0
                            if r0 < 0:
                                r0, rows, out_off = 0, CHUNK - 1, W
                            elif r0 + CHUNK > H:
                                rows = H - r0
                            for kx in range(3):
                                t = 3 * ky + kx
                                nc.tensor.matmul(
                                    ps[:, out_off : out_off + rows * W],
                                    w_sb[:, t * COUT : (t + 1) * COUT],
                                    x_t[:, r0 : r0 + rows, kx : kx + W],
                                    start=first, stop=(t == 8),
                                )
                                first = False
                        nc.vector.tensor_scalar_add(
                            out_sb[:, q * CHUNK * W : (q + 1) * CHUNK * W],
                            ps, bias_sb,
                        )
                    # border corrections (replicate-pad delta)
                    v = out_sb.rearrange("p (r c) -> p r c", c=W)
                    r0 = blk * ROWS_PER_BLK
                    r1 = r0 + ROWS_PER_BLK
                    cl = corr_sb[:, r0:r1].rearrange("p (r c) -> p r c", c=1)
                    cr = corr_sb[:, H + r0 : H + r1].rearrange("p (r c) -> p r c", c=1)
                    nc.vector.tensor_sub(v[:, :, 0:1], v[:, :, 0:1], cl)
                    nc.vector.tensor_sub(v[:, :, W - 1 : W], v[:, :, W - 1 : W], cr)
                    if blk == 0:
                        nc.vector.tensor_sub(
                            out_sb[:, 0:W], out_sb[:, 0:W],
                            corr_sb[:, 2 * H : 2 * H + W],
                        )
                    if blk == NBLK - 1:
                        last = (ROWS_PER_BLK - 1) * W
                        nc.vector.tensor_sub(
                            out_sb[:, last : last + W], out_sb[:, last : last + W],
                            corr_sb[:, 2 * H + W :],
                        )
                    nc.sync.dma_start(
                        out=y_ext[img, :, r0:r1, :],
                        in_=out_sb.rearrange("p (r c) -> p r c", c=W),
                    )
    _split_excess_waits(nc)
    return nc


def _get_runner(compute, reps=1):
    """Compile once per process; returns (fn, in_names, out_names, shapes),
    sharded over the 8 cores."""
    key = (compute, reps)
    if key in _runner:
        return _runner[key]

    import jax
    import jax.numpy as jnp
    from jax.sharding import Mesh, PartitionSpec
    from jax.experimental.shard_map import shard_map
    import concourse.mybir as mybir
    from concourse import bass2jax

    if key not in _built:
        _built[key] = _build(compute, reps)
    nc = _built[key]

    bass2jax.install_neuronx_cc_hook()

    partition_name = (
        nc.partition_id_tensor.name if nc.partition_id_tensor else None
    )
    in_names, out_names, out_avals, zero_shapes = [], [], [], []
    for alloc in nc.m.functions[0].allocations:
        if not isinstance(alloc, mybir.MemoryLocationSet):
            continue
        name = alloc.memorylocations[0].name
        if alloc.kind == "ExternalInput":
            if name != partition_name:
                in_names.append(name)
        elif alloc.kind == "ExternalOutput":
            out_names.append(name)
            shape = tuple(alloc.tensor_shape)
            dtype = mybir.dt.np(alloc.dtype)
            out_avals.append(jax.core.ShapedArray(shape, dtype))
            zero_shapes.append((shape, dtype))
    n_params = len(in_names)
    all_in_names = in_names + out_names
    if partition_name is not None:
        all_in_names.append(partition_name)
    donate = tuple(range(n_params, n_params + len(out_names)))

    def _body(*args):
        operands = list(args)
        if partition_name is not None:
            operands.append(bass2jax.partition_id_tensor())
        outs = bass2jax._bass_exec_p.bind(
            *operands,
            out_avals=tuple(out_avals),
            in_names=tuple(all_in_names),
            out_names=tuple(out_names),
            lowering_input_output_aliases=(),
            sim_require_finite=True,
            sim_require_nnan=True,
            nc=nc,
        )
        return tuple(outs)

    devices = jax.devices()[:N_CORES]
    mesh = Mesh(np.asarray(devices), ("core",))
    nio = n_params + len(out_names)
    sharded = jax.jit(
        shard_map(
            _body, mesh=mesh,
            in_specs=(PartitionSpec("core"),) * nio,
            out_specs=(PartitionSpec("core"),) * len(out_names),
            check_rep=False,
        ),
        donate_argnums=donate, keep_unused=True,
    )
    _runner[key] = (sharded, in_names, out_names, zero_shapes)
    return _runner[key]


def _prep_inputs(x, Wm, b, compute):
    import ml_dtypes

    cross = np.array([[0, 1, 0], [1, -4, 1], [0, 1, 0]], np.float32)
    Wf = np.asarray(Wm, np.float32) - THETA * cross[None, None]
    Wt = np.ascontiguousarray(Wf.transpose(1, 2, 3, 0)).reshape(CIN, 9 * COUT)
    npdt = np.float32 if compute == "f32r" else ml_dtypes.bfloat16
    xp = np.zeros((B, CIN, H, PW), npdt)
    xp[:, :, :, 1 : W + 1] = np.asarray(x)
    Wts = np.ascontiguousarray(Wt.astype(npdt, copy=False))
    bs = np.ascontiguousarray(np.asarray(b, np.float32).reshape(COUT, 1))
    ones = np.ones((CIN, 1), npdt)
    # global (concat over cores along axis 0) arrays for shard_map
    feed = {
        "x": xp,
        "Wt": np.concatenate([Wts[None]] * N_CORES, 0).reshape(N_CORES * CIN, 9 * COUT),
        "bias": np.concatenate([bs[None]] * N_CORES, 0).reshape(N_CORES * COUT, 1),
        "ones": np.concatenate([ones[None]] * N_CORES, 0).reshape(N_CORES * CIN, 1),
    }
    return feed


def _run(x, Wm, b, compute):
    import jax.numpy as jnp

    sharded, in_names, out_names, zero_shapes = _get_runner(compute)
    feed = _prep_inputs(x, Wm, b, compute)
    ins = [feed[n] for n in in_names]
    zeros = [
        jnp.zeros((N_CORES * s[0], *s[1:]), d) for (s, d) in zero_shapes
    ]
    outs = sharded(*ins, *zeros)
    y = np.asarray(outs[out_names.index("y")])
    return y.reshape(B, COUT, H, W).astype(np.float32, copy=False)


def kernel(x, W, b):
    try:
        return _run(x, W, b, COMPUTE)
    except Exception:
        # one retry: transient device/terminal hiccups (e.g. a wedged core
        # from a previous session) usually clear on re-execution
        import time

        time.sleep(5.0)
        return _run(x, W, b, COMPUTE)



# revision 4
# speedup vs baseline: 152.8997x; 152.8997x over previous
"""Trainium2 Bass/Tile kernel: 3x3 conv (zero pad) + bias - theta * cross-stencil
(replicate pad) over NCHW f32, B=32, Cin=Cout=128, H=W=128, theta=0.7.

Math: the stencil term is a 3x3 conv with kernel [[0,1,0],[1,-4,1],[0,1,0]]
applied to sum_ci(x), identical for all (out,in) channel pairs.  For interior
pixels it folds into the conv weights:  W' = W - theta*cross.  The only
difference is at the 1-pixel image border where the stencil uses replicate
padding (out-of-bounds neighbor == edge value) while the conv uses zero
padding.  So:  out = conv_zp(x, W') + b - theta*corr, where corr adds
s=sum_ci(x) at each border pixel once per out-of-bounds neighbor (corners
twice).  corr is computed on-device from four border-strip channel sums
(ones-vector matmuls) broadcast across the 128 output channels.

Sharding: data-parallel over batch, 4 images per core, 8 cores, SPMD.
"""

import os
import numpy as np

THETA = 0.7
N_CORES = 8
B, CIN, COUT, H, W = 32, 128, 128, 128, 128
BL = B // N_CORES          # images per core
PW = W + 2                 # horizontally padded row width in SBUF
ROWS_PER_BLK = 32          # output rows per SBUF staging tile / output DMA
CHUNK = 4                  # output rows per PSUM accumulation group (N=512)
XGRP = 4                   # x-image DMA split into XGRP row-groups

COMPUTE = os.environ.get("BASS_CONV_DTYPE", "bf16")  # "f32r" | "bf16"

_built = {}
_runner = {}


def _patch_tile_drain():
    """This toolchain's walrus rejects instructions carrying more than one
    semaphore wait ('Too many sync wait commands' in setupSyncWait).  Tile's
    exit drain accumulates one wait per live semaphore on a single Drain, so
    re-emit those waits as a chain of single-wait NOPs in front of it."""
    import concourse.tile as tile
    import concourse.mybir as mybir
    from concourse.vector_clock import ScopedClock

    if getattr(tile.TileContext, "_drain_patched", False):
        return

    def _drain_and_barrier(self, tick_clock, wait_clock):
        nc = self.nc
        probe = nc.sync.nop(nofuse=True)
        wait_clock.add_sem_waits(
            probe.ins, ScopedClock({None: tick_clock.global_clock})
        )
        si = probe.ins.sync_info
        waits = list(si.on_wait) if si is not None and si.on_wait else []
        if len(waits) > 1:
            si.on_wait = waits[:1]
            for w in waits[1:]:
                nop = nc.sync.nop(nofuse=True)
                if nop.ins.sync_info is None:
                    nop.ins.sync_info = mybir.SyncInfo(on_wait=[w], on_update=[])
                else:
                    nop.ins.sync_info.on_wait = [w]
        nc.sync.drain()

        nc.all_engine_barrier()
        assert self.sems is not None
        popped = nc._tile_sem_poison_stack.pop()
        assert popped is self._sem_poison
        nc.clear_and_free_semaphores(list(self.sems.allocated().values()))
        nc.all_engine_barrier()

    tile.TileContext._drain_and_barrier = _drain_and_barrier
    tile.TileContext._drain_patched = True


def _split_excess_waits(nc, cap=1):
    """Hoist extra semaphore waits (walrus allows only `cap` per instruction
    on this toolchain) onto same-engine NOPs inserted just before the
    offending instruction."""
    import concourse.mybir as mybir

    n = 0
    for bb in nc.main_func.blocks:
        insts = bb.instructions
        out = []
        for inst in insts:
            si = inst.sync_info
            waits = list(si.on_wait) if si is not None and si.on_wait else []
            if len(waits) > cap:
                n += 1
                for i in range(0, len(waits) - cap, cap):
                    chunk = waits[i : i + cap]
                    nop = mybir.InstNoOp(
                        name=nc.get_next_instruction_name(),
                        sync_info=mybir.SyncInfo(on_wait=list(chunk), on_update=[]),
                        engine=inst.engine,
                        bass_nofuse=True,
                    )
                    nc.register_instruction(nop)
                    out.append(nop)
                si.on_wait = waits[len(waits) - cap :]
            out.append(inst)
        insts[:] = out
    return n


def _build(compute, reps=1):
    import concourse.bass as bass
    import concourse.mybir as mybir
    import concourse.tile as tile

    _patch_tile_drain()
    cdt = {"f32r": mybir.dt.float32r, "bf16": mybir.dt.bfloat16}[compute]
    f32 = mybir.dt.float32
    AF = mybir.ActivationFunctionType

    nc = bass.Bass()
    # x arrives pre-padded on the host: two zero columns per row (PW=W+2),
    # so the image DMA is one contiguous copy and zero-padding needs no
    # on-device memsets (f32r memset fails the walrus ISA check).
    x_ext = nc.declare_dram_parameter("x", [BL, CIN, H, PW], cdt, isOutput=False)
    wt_ext = nc.declare_dram_parameter("Wt", [CIN, 9 * COUT], cdt, isOutput=False)
    b_ext = nc.declare_dram_parameter("bias", [COUT, 1], f32, isOutput=False)
    ones_ext = nc.declare_dram_parameter("ones", [CIN, 1], cdt, isOutput=False)
    y_ext = nc.declare_dram_parameter("y", [BL, COUT, H, W], f32, isOutput=True)

    NBLK = H // ROWS_PER_BLK
    NCH = ROWS_PER_BLK // CHUNK

    with tile.TileContext(nc) as tc:
        with (
            tc.tile_pool(name="singles", bufs=1) as singles,
            tc.tile_pool(name="xin", bufs=3 if compute == "bf16" else 2) as xpool,
            tc.tile_pool(name="outs", bufs=3) as outpool,
            tc.tile_pool(name="corr", bufs=2) as corrpool,
            tc.tile_pool(name="pmm", bufs=4, space="PSUM") as pmm,
            tc.tile_pool(name="psmall", bufs=2, space="PSUM") as psmall,
            tc.tile_pool(name="pbcast", bufs=2, space="PSUM") as pbcast,
        ):
            w_sb = singles.tile([CIN, 9 * COUT], cdt)
            nc.sync.dma_start(out=w_sb, in_=wt_ext[:])
            bias_sb = singles.tile([COUT, 1], f32)
            nc.sync.dma_start(out=bias_sb, in_=b_ext[:])
            ones_col = singles.tile([CIN, 1], cdt)
            nc.sync.dma_start(out=ones_col, in_=ones_ext[:])
            ones_row = singles.tile([1, COUT], f32)
            nc.vector.memset(ones_row, 1.0)

            for img_rep in range(BL * reps):
                img = img_rep % BL
                x_t = xpool.tile([CIN, H, PW], cdt)
                # split the image DMA into row-groups so conv chunks for the
                # first rows can start before the whole image has landed
                gr = H // XGRP
                for g in range(XGRP):
                    nc.sync.dma_start(
                        out=x_t[:, g * gr : (g + 1) * gr],
                        in_=x_ext[img, :, g * gr : (g + 1) * gr],
                    )

                # Channel sums of the four border strips -> one PSUM bank:
                # [0:H) left col, [H:2H) right col, [2H:2H+W) top, [2H+W:) bottom
                ps_s = psmall.tile([1, 2 * H + 2 * W], f32)
                nc.tensor.matmul(
                    ps_s[:, 0:H], ones_col, x_t[:, :, 1:2], start=True, stop=False
                )
                nc.tensor.matmul(
                    ps_s[:, H : 2 * H], ones_col, x_t[:, :, W : W + 1],
                    start=False, stop=False,
                )
                nc.tensor.matmul(
                    ps_s[:, 2 * H : 2 * H + W], ones_col, x_t[:, 0:1, 1 : W + 1],
                    start=False, stop=False,
                )
                nc.tensor.matmul(
                    ps_s[:, 2 * H + W :], ones_col, x_t[:, H - 1 : H, 1 : W + 1],
                    start=False, stop=True,
                )
                s_sb = corrpool.tile([1, 2 * H + 2 * W], f32)
                nc.scalar.activation(out=s_sb, in_=ps_s, func=AF.Copy)
                # broadcast across the 128 out-channel partitions, scaled by theta
                ps_c = pbcast.tile([COUT, 2 * H + 2 * W], f32)
                nc.tensor.matmul(ps_c, ones_row, s_sb, start=True, stop=True)
                corr_sb = corrpool.tile([COUT, 2 * H + 2 * W], f32)
                nc.scalar.activation(out=corr_sb, in_=ps_c, func=AF.Copy, scale=THETA)

                for blk in range(NBLK):
                    out_sb = outpool.tile([COUT, ROWS_PER_BLK * W], f32)
                    for q in range(NCH):
                        y0 = blk * ROWS_PER_BLK + q * CHUNK
                        ps = pmm.tile([COUT, CHUNK * W], f32)
                        first = True
                        for ky in range(3):
                            r0 = y0 + ky - 1
                            rows, out_off = CHUNK, 0
                            if r0 < 0:
                                r0, rows, out_off = 0, CHUNK - 1, W
                            elif r0 + CHUNK > H:
                                rows = H - r0
                            for kx in range(3):
                                t = 3 * ky + kx
                                nc.tensor.matmul(
                                    ps[:, out_off : out_off + rows * W],
                                    w_sb[:, t * COUT : (t + 1) * COUT],
                                    x_t[:, r0 : r0 + rows, kx : kx + W],
                                    start=first, stop=(t == 8),
                                )
                                first = False
                        nc.vector.tensor_scalar_add(
                            out_sb[:, q * CHUNK * W : (q + 1) * CHUNK * W],
                            ps, bias_sb,
                        )
                    # border corrections (replicate-pad delta)
                    v = out_sb.rearrange("p (r c) -> p r c", c=W)
                    r0 = blk * ROWS_PER_BLK
                    r1 = r0 + ROWS_PER_BLK
                    cl = corr_sb[:, r0:r1].rearrange("p (r c) -> p r c", c=1)
                    cr = corr_sb[:, H + r0 : H + r1].rearrange("p (r c) -> p r c", c=1)
                    nc.vector.tensor_sub(v[:, :, 0:1], v[:, :, 0:1], cl)
                    nc.vector.tensor_sub(v[:, :, W - 1 : W], v[:, :, W - 1 : W], cr)
                    if blk == 0:
                        nc.vector.tensor_sub(
                            out_sb[:, 0:W], out_sb[:, 0:W],
                            corr_sb[:, 2 * H : 2 * H + W],
                        )
                    if blk == NBLK - 1:
                        last = (ROWS_PER_BLK - 1) * W
                        nc.vector.tensor_sub(
                            out_sb[:, last : last + W], out_sb[:, last : last + W],
                            corr_sb[:, 2 * H + W :],
                        )
                    nc.sync.dma_start(
                        out=y_ext[img, :, r0:r1, :],
                        in_=out_sb.rearrange("p (r c) -> p r c", c=W),
                    )
    _split_excess_waits(nc)
    return nc


def _get_runner(compute, reps=1):
    """Compile once per process; returns (fn, in_names, out_names, shapes),
    sharded over the 8 cores."""
    key = (compute, reps)
    if key in _runner:
        return _runner[key]

    import jax
    import jax.numpy as jnp
    from jax.sharding import Mesh, PartitionSpec
    from jax.experimental.shard_map import shard_map
    import concourse.mybir as mybir
    from concourse import bass2jax

    if key not in _built:
        _built[key] = _build(compute, reps)
    nc = _built[key]

    bass2jax.install_neuronx_cc_hook()

    partition_name = (
        nc.partition_id_tensor.name if nc.partition_id_tensor else None
    )
    in_names, out_names, out_avals, zero_shapes = [], [], [], []
    for alloc in nc.m.functions[0].allocations:
        if not isinstance(alloc, mybir.MemoryLocationSet):
            continue
        name = alloc.memorylocations[0].name
        if alloc.kind == "ExternalInput":
            if name != partition_name:
                in_names.append(name)
        elif alloc.kind == "ExternalOutput":
            out_names.append(name)
            shape = tuple(alloc.tensor_shape)
            dtype = mybir.dt.np(alloc.dtype)
            out_avals.append(jax.core.ShapedArray(shape, dtype))
            zero_shapes.append((shape, dtype))
    n_params = len(in_names)
    all_in_names = in_names + out_names
    if partition_name is not None:
        all_in_names.append(partition_name)
    donate = tuple(range(n_params, n_params + len(out_names)))

    def _body(*args):
        operands = list(args)
        if partition_name is not None:
            operands.append(bass2jax.partition_id_tensor())
        outs = bass2jax._bass_exec_p.bind(
            *operands,
            out_avals=tuple(out_avals),
            in_names=tuple(all_in_names),
            out_names=tuple(out_names),
            lowering_input_output_aliases=(),
            sim_require_finite=True,
            sim_require_nnan=True,
            nc=nc,
        )
        return tuple(outs)

    devices = jax.devices()[:N_CORES]
    mesh = Mesh(np.asarray(devices), ("core",))
    nio = n_params + len(out_names)
    sharded = jax.jit(
        shard_map(
            _body, mesh=mesh,
            in_specs=(PartitionSpec("core"),) * nio,
            out_specs=(PartitionSpec("core"),) * len(out_names),
            check_rep=False,
        ),
        donate_argnums=donate, keep_unused=True,
    )
    _runner[key] = (sharded, in_names, out_names, zero_shapes)
    return _runner[key]


def _prep_inputs(x, Wm, b, compute):
    import ml_dtypes

    cross = np.array([[0, 1, 0], [1, -4, 1], [0, 1, 0]], np.float32)
    Wf = np.asarray(Wm, np.float32) - THETA * cross[None, None]
    Wt = np.ascontiguousarray(Wf.transpose(1, 2, 3, 0)).reshape(CIN, 9 * COUT)
    npdt = np.float32 if compute == "f32r" else ml_dtypes.bfloat16
    xp = np.zeros((B, CIN, H, PW), npdt)
    xp[:, :, :, 1 : W + 1] = np.asarray(x)
    Wts = np.ascontiguousarray(Wt.astype(npdt, copy=False))
    bs = np.ascontiguousarray(np.asarray(b, np.float32).reshape(COUT, 1))
    ones = np.ones((CIN, 1), npdt)
    # global (concat over cores along axis 0) arrays for shard_map
    feed = {
        "x": xp,
        "Wt": np.concatenate([Wts[None]] * N_CORES, 0).reshape(N_CORES * CIN, 9 * COUT),
        "bias": np.concatenate([bs[None]] * N_CORES, 0).reshape(N_CORES * COUT, 1),
        "ones": np.concatenate([ones[None]] * N_CORES, 0).reshape(N_CORES * CIN, 1),
    }
    return feed


def _run(x, Wm, b, compute):
    import jax.numpy as jnp

    sharded, in_names, out_names, zero_shapes = _get_runner(compute)
    feed = _prep_inputs(x, Wm, b, compute)
    ins = [feed[n] for n in in_names]
    zeros = [
        jnp.zeros((N_CORES * s[0], *s[1:]), d) for (s, d) in zero_shapes
    ]
    outs = sharded(*ins, *zeros)
    y = np.asarray(outs[out_names.index("y")])
    return y.reshape(B, COUT, H, W).astype(np.float32, copy=False)


def kernel(x, W, b):
    try:
        return _run(x, W, b, COMPUTE)
    except Exception:
        # one retry: transient device/terminal hiccups (e.g. a wedged core
        # from a previous session) usually clear on re-execution
        import time

        time.sleep(5.0)
        return _run(x, W, b, COMPUTE)

